# revision 9
# baseline (speedup 1.0000x reference)
"""QRNN forget-mult recurrence h_t = i_t*z_t + f_t*h_{t-1} on 8 NeuronCores.

Sharding: batch dim B=32 split 4-per-core (data parallel). Per core the
[T=4096, B=4, H=256] slice is viewed as C=1024 channels, staged host-side
into channel-major [C, T].

The kernel is HBM-bandwidth bound, so f and i (both in [0,1)) ship as
uint8 fixed-point (abs err <= 1/510) and z as fp16: 24 MiB/core instead
of the fp32 baseline's 64 or the all-fp16 32. The i dequant scale is
folded into the output domain instead of a device op: the scan computes
H = 255*h via H_t = f_t*H_{t-1} + i_q*z_t (raw u8 x f16 product <= 1382,
H <= 255*|h|max ~ 7.5e3, both far inside f16 range), and the host
divides by 255 during unshard. Scale-relative absmax err ~6e-3 vs the
2e-2 gate; the scan's fp32 internal state keeps rounding from
compounding.

Engine placement (one op per engine, no SBUF port contention - the
DVE<->GpSimd shared port only arbitrates DVE *perf-mode* ops, and the
scan uses DVE's dedicated port): f-dequant on Act (activation Copy,
scale=1/255), iz = i_q * z on Pool, TensorTensorScan on DVE. DMA
queues: f,i loads + z-half + 2 stores on SP, z-half + 1 store on Act
(under its dequant), 5 stores on Pool.
"""

import numpy as np

T = 4096
B = 32
H = 256
NCORES = 8
BS = B // NCORES          # batches per core
C = BS * H                # channels per core
P = 128                   # partitions
NG = C // P               # channel groups per core

_CACHE = {}


def _build_nc(sc=T, repeat=1, t_len=None):
    import concourse.tile as tile
    from concourse import bacc, mybir

    f16 = mybir.dt.float16
    f32 = mybir.dt.float32
    u8 = mybir.dt.uint8
    mult = mybir.AluOpType.mult
    add = mybir.AluOpType.add
    copy_fn = mybir.ActivationFunctionType.Copy
    TT = T if t_len is None else t_len
    sc = min(sc, TT)
    nchunks = TT // sc

    nc = bacc.Bacc("TRN2", target_bir_lowering=False, debug=False)
    f_d = nc.dram_tensor("f", [C, TT], u8, kind="ExternalInput")
    i_d = nc.dram_tensor("i", [C, TT], u8, kind="ExternalInput")
    z_d = nc.dram_tensor("z", [C, TT], f16, kind="ExternalInput")
    h0_d = nc.dram_tensor("h0", [C, 1], f32, kind="ExternalInput")
    h_d = nc.dram_tensor("h", [C, TT], f16, kind="ExternalOutput")

    with tile.TileContext(nc) as tc:
        with (
            tc.tile_pool(name="const", bufs=1) as constp,
            tc.tile_pool(name="ins", bufs=4) as insp,
            tc.tile_pool(name="deq", bufs=3) as deqp,
            tc.tile_pool(name="izp", bufs=3) as izp,
            tc.tile_pool(name="hts", bufs=3) as htp,
        ):
            h0t = constp.tile([P, NG], f32)
            nc.sync.dma_start(h0t[:], h0_d.rearrange("(g p) o -> p (g o)", p=P))

            for rep in range(repeat):
                for g in range(NG):
                    gs = slice(g * P, (g + 1) * P)
                    ft = insp.tile([P, TT], u8, tag="f")
                    nc.sync.dma_start(ft[:], f_d[gs, :])
                    it = insp.tile([P, TT], u8, tag="i")
                    nc.sync.dma_start(it[:], i_d[gs, :])
                    zt = insp.tile([P, TT], f16, tag="z")
                    z_eng = nc.sync if g % 2 == 0 else nc.scalar
                    z_eng.dma_start(zt[:], z_d[gs, :])
                    f16t = deqp.tile([P, TT], f16, tag="f16")
                    nc.scalar.activation(f16t[:], ft[:], copy_fn,
                                         scale=1.0 / 255.0)
                    izt = izp.tile([P, TT], f16, tag="iz")
                    nc.gpsimd.tensor_mul(izt[:], it[:], zt[:])
                    ht = htp.tile([P, TT], f16, tag="h")
                    st_eng = (nc.sync, nc.sync, nc.scalar, nc.gpsimd,
                              nc.gpsimd, nc.gpsimd, nc.gpsimd, nc.gpsimd)[g]
                    for k in range(nchunks):
                        ts = slice(k * sc, (k + 1) * sc)
                        init = h0t[:, g:g + 1] if k == 0 else \
                            ht[:, k * sc - 1:k * sc]
                        nc.vector.tensor_tensor_scan(ht[:, ts], f16t[:, ts],
                                                     izt[:, ts], init,
                                                     op0=mult, op1=add)
                        st_eng.dma_start(h_d[gs, ts], ht[:, ts])

    nc.compile()
    return nc


def _get_nc():
    if "nc" not in _CACHE:
        _CACHE["nc"] = _build_nc()
    return _CACHE["nc"]


def make_in_maps(f, z, i, hidden_init):
    # host-side staging: quantize f,i to uint8 fixed point (q = round(x*255),
    # dequant f = q/255 on device; i's 1/255 is folded into the H = 255*h
    # output domain), z to fp16, and transpose [T, B, H] -> [B, H, T] so each
    # core's slice is a contiguous channel-major [C, T] view. The scan init
    # is 255*h0 to match the H domain.
    fq = np.round(np.asarray(f, np.float32) * np.float32(255.0)).astype(np.uint8)
    iq = np.round(np.asarray(i, np.float32) * np.float32(255.0)).astype(np.uint8)
    z16 = np.asarray(z, np.float16)

    fT = np.ascontiguousarray(fq.transpose(1, 2, 0))
    iT = np.ascontiguousarray(iq.transpose(1, 2, 0))
    zT = np.ascontiguousarray(z16.transpose(1, 2, 0))
    h0 = np.asarray(hidden_init, np.float32) * np.float32(255.0)
    in_maps = []
    for c in range(NCORES):
        b0 = c * BS
        in_maps.append({
            "f": fT[b0:b0 + BS].reshape(C, T),
            "i": iT[b0:b0 + BS].reshape(C, T),
            "z": zT[b0:b0 + BS].reshape(C, T),
            "h0": np.ascontiguousarray(h0[b0:b0 + BS]).reshape(C, 1),
        })
    return in_maps


def unshard(h_list):
    """Per-core [C, T] fp16 channel-major H=255h outputs -> [T, B, H] fp32 h."""
    hT = np.empty((B, H, T), np.float16)
    for c in range(NCORES):
        hT[c * BS:(c + 1) * BS] = np.asarray(h_list[c]).reshape(BS, H, T)
    return hT.transpose(2, 0, 1).astype(np.float32) * np.float32(1.0 / 255.0)


def kernel(f, z, i, hidden_init):
    import time

    from concourse.bass_utils import run_bass_kernel_spmd

    in_maps = make_in_maps(f, z, i, hidden_init)
    last_err = None
    for attempt in range(3):
        try:
            res = run_bass_kernel_spmd(
                _get_nc(), in_maps, list(range(NCORES))
            ).results
            break
        except Exception as e:  # transient device-unrecoverable states
            last_err = e
            time.sleep(2.0 * (attempt + 1))
    else:
        raise last_err
    return unshard([res[c]["h"] for c in range(NCORES)])
